# revision 33
# baseline (speedup 1.0000x reference)
"""AdaConv (low-rank dynamic conv) on 8 trn2 NeuronCores.

out[b,o,h,w] = sum_e para[b,e,h,w] * conv3x3(x, W_e)[b,o,h,w]
  para = conv3x3(relu(conv1x1(x, pw) + pb), cw) + cb          (16 bases)
  W_e  = basis weight e reshaped from W (64 out, 64 in, 3x3)

Sharding: pure data parallel, 8 shards = (batch b, image half hh).
Each core computes out rows [hh*64, hh*64+64) of image b from x rows
[hh*64-1, hh*64+65) (1-row halo), zero padded at image borders.

Per-core kernel (pixel-partition layout, im2col fully by view):
  - host sends x2 (128, 66*130) bf16: rows 0-63 x (width-padded to 130),
    rows 64-127 the same image shifted by +130 (one padded row). A K=128
    matmul view at column offset o contracts taps (o-131, o-1) at once:
    offsets 0/1/2 cover the dh=-1 and dh=0 tap rows (6 taps, full K).
  - the dh=+1 taps (129/130/131) use K=64 row-tiled matmul PAIRS running
    concurrently on disjoint PE row groups: top rows (x) at offset
    131+tap serve one psum half while bottom rows (x shifted) at offset
    tap+1 serve the other; cross-pairing keeps both halves fed each
    slot, so 3 slots finish all 3 taps with no half-zero weight waste
    and no DMA-materialized im2col chunk.
  - conv1 also runs as row-tiled concurrent pairs (top half processes
    columns [a,a+512), bottom half [a+512,a+1024) via the +130 copy).
  - conv2 im2col: pc8 (8 taps x 16ch) + pt8 (ones row + tap8) of pb1,
    built by shifted SBUF->SBUF DMAs in 5 bands chasing conv1.
  - per output row h (128 pixels): z psum (128pix, 1024 eo) = 6 full-K
    matmuls + 3 row-tiled pair slots; para psum (128pix,16) from 2
    matmuls; epilogue multiplies z e-slabs by para columns (ScalarE
    2 slabs + VectorE 14) and tree-adds to bf16 out.
  - tiles run in FUSED PAIRS (full-K runs, conv2 smalls, and row-tiled
    runs each contiguous across both tiles): the PE charges ~95ns of
    drain serialization at every full<->row-tiled regime change, so
    pairing halves that cost; 3-tile groups exhaust the 6 z psum banks
    and stall.  Inputs stream on one queue in first-use order, z for
    tiles 0-1 is emitted before conv1 (whose late pairs wait on x2
    chunks), and 7 zero matmuls bridge the PE's HAM clock-gate from the
    preamble to the first real matmul so the stream starts at 2.4 GHz.
  - out written pixel-major (8192, 64) bf16; host transposes + casts.
"""

import numpy as np
import ml_dtypes

import concourse.bass as bass
import concourse.mybir as mybir
import concourse.tile as tile
from concourse import bacc
from concourse.bass_utils import run_bass_kernel_spmd

BF16 = ml_dtypes.bfloat16

B, C, H, WD = 4, 64, 128, 128
E = 16            # bases
NCORES = 8
HALF = H // 2     # 64 output rows per core
RH = HALF + 2     # 66 stored x rows (1 halo each side)
WP = WD + 2       # 130 padded width
L = RH * WP       # 8580 columns of the padded per-core image
LC = L - 2 * (WP + 1)  # 8318: im2col span, col i <-> center index i+131
NT = HALF         # 64 row-tiles per core, 128 pixels each
NACT = 2          # e-slabs handled by ScalarE in the epilogue
# tap order within a 3x3 kernel: t = (dh+1)*3 + (dw+1), offset = dh*130+dw

XCUTS = [0, 512, 1536, 3584, 5632, L]       # x2 DMA chunks (geometric-ish)
PCUTS = [0, 600, 2160, 4240, 6320, LC]      # pc8/pt8 build bands


def _prep_weights(Wt, pw, pb, cw, cb):
    """Host-side relayout of all weights (small, replicated to all cores)."""
    T = np.asarray(Wt, np.float32).reshape(C, C, 9, E)   # [o, c, tap, e]
    A = T.transpose(2, 1, 3, 0).reshape(9, C, E * C)     # [tap, c, (e*64+o)]
    HN = E * C // 2  # 512
    wm = {
        # full-K chunks: view o covers taps (o-131 via top, o-1 via bottom)
        "wm0": np.concatenate([A[0], A[3]], axis=0),   # taps -131, -1
        "wm1": np.concatenate([A[1], A[4]], axis=0),   # taps -130, 0
        "wm2": np.concatenate([A[2], A[5]], axis=0),   # taps -129, 1
        # row-tiled pair weights: [0:64]=top-row weights (half0 cols),
        # [64:128]=bottom-row weights (half1 cols)
        "wmP": np.concatenate([A[6][:, :HN], A[7][:, HN:]], axis=0),
        "wmQ": np.concatenate([A[7][:, :HN], A[6][:, HN:]], axis=0),
        "wmR": np.concatenate([A[8][:, :HN], A[8][:, HN:]], axis=0),
    }
    pwm = np.asarray(pw, np.float32).reshape(E, C).T          # (64, 16)
    pwm2 = np.concatenate([pwm, pwm], axis=0)                 # (128, 16)
    pbv = np.asarray(pb, np.float32).reshape(E, 1)            # (16, 1) f32

    A2 = np.asarray(cw, np.float32).transpose(2, 3, 1, 0).reshape(9, E, E)
    cw8 = A2[:8].reshape(8 * E, E)                            # (128, 16)
    # row 0 = cb (pairs with the ones row at partition 0 of pt8)
    cw1 = np.concatenate(
        [np.asarray(cb, np.float32).reshape(1, E), A2[8]], axis=0)  # (17, 16)

    out = {k: v.astype(BF16) for k, v in wm.items()}
    out.update(pwm2=pwm2.astype(BF16), pbv=np.asarray(pbv, np.float32),
               cw8=cw8.astype(BF16), cw1=cw1.astype(BF16))
    return out


def _shard_x(x):
    """(B,C,H,W) f32 -> 8 shards (128, L) bf16: padded image + copy at +130."""
    xp = np.zeros((B, C, H + 2, WP), np.float32)
    xp[:, :, 1:H + 1, 1:WD + 1] = x
    shards = []
    for b in range(B):
        for hh in range(2):
            rows = xp[b, :, hh * HALF: hh * HALF + RH, :].reshape(C, L)
            shifted = np.zeros_like(rows)
            shifted[:, :L - WP] = rows[:, WP:]
            shards.append(np.concatenate([rows, shifted], axis=0).astype(BF16))
    return shards


def build_core_inputs(inputs):
    """Full inputs dict -> list of per-core in_maps (host relayout)."""
    wts = _prep_weights(inputs["W"], inputs["pw"], inputs["pb"],
                        inputs["cw"], inputs["cb"])
    shards = _shard_x(np.asarray(inputs["x"], np.float32))
    return [dict(wts, x=shards[i]) for i in range(NCORES)]


def unshard_output(results):
    """Per-core result dicts -> full (B, C, H, W) f32 output."""
    out = np.empty((B, C, H, WD), np.float32)
    for i in range(NCORES):
        b, hh = divmod(i, 2)
        sh = np.asarray(results[i]["out"], np.float32)  # (8192, 64) pixel-major
        out[b, :, hh * HALF:(hh + 1) * HALF, :] = (
            sh.reshape(HALF, WD, C).transpose(2, 0, 1))
    return out


def build_bass():
    f32 = mybir.dt.float32
    bf16 = mybir.dt.bfloat16
    Relu = mybir.ActivationFunctionType.Relu
    Copy = mybir.ActivationFunctionType.Copy
    HN = E * C // 2  # 512

    nc = bacc.Bacc("TRN2", target_bir_lowering=False, debug=False,
                   num_devices=NCORES)

    x_d = nc.declare_dram_parameter("x", [2 * C, L], bf16, isOutput=False)
    wmF_d = [nc.declare_dram_parameter(f"wm{k}", [2 * C, E * C], bf16,
                                       isOutput=False) for k in range(3)]
    wmP_d = nc.declare_dram_parameter("wmP", [2 * C, HN], bf16, isOutput=False)
    wmQ_d = nc.declare_dram_parameter("wmQ", [2 * C, HN], bf16, isOutput=False)
    wmR_d = nc.declare_dram_parameter("wmR", [2 * C, HN], bf16, isOutput=False)
    pwm2_d = nc.declare_dram_parameter("pwm2", [2 * C, E], bf16, isOutput=False)
    pbv_d = nc.declare_dram_parameter("pbv", [E, 1], f32, isOutput=False)
    cw8_d = nc.declare_dram_parameter("cw8", [8 * E, E], bf16, isOutput=False)
    cw1_d = nc.declare_dram_parameter("cw1", [E + 1, E], bf16, isOutput=False)
    out_d = nc.declare_dram_parameter("out", [HALF * WD, C], bf16,
                                      isOutput=True)

    with tile.TileContext(nc) as tc:
        with (
            tc.tile_pool(name="const", bufs=1) as constp,
            tc.tile_pool(name="big", bufs=1) as bigp,
            tc.tile_pool(name="work", bufs=3) as workp,
            tc.tile_pool(name="ps_z", bufs=6, space="PSUM") as zpool,
            tc.tile_pool(name="ps_p", bufs=2, space="PSUM") as ppool,
        ):
            # --- input DMAs: one queue (sync) in first-use order so the
            # DMA engine pool delivers the z-critical path first ---
            x2 = bigp.tile([2 * C, L], bf16, tag="x2")
            pwm2 = constp.tile([2 * C, E], bf16, tag="pwm2")
            nc.sync.dma_start(pwm2[:], pwm2_d.ap())
            pbv = constp.tile([E, 1], f32, tag="pbv")
            nc.sync.dma_start(pbv[:], pbv_d.ap())
            nc.sync.dma_start(x2[:, XCUTS[0]:XCUTS[1]],
                              x_d.ap()[:, XCUTS[0]:XCUTS[1]])
            wmF = []
            for k in range(3):
                t = constp.tile([2 * C, E * C], bf16,
                                name=f"wm{k}s", tag=f"wm{k}")
                nc.sync.dma_start(t[:], wmF_d[k].ap())
                wmF.append(t)
            wmP = constp.tile([2 * C, HN], bf16, tag="wmP")
            nc.sync.dma_start(wmP[:], wmP_d.ap())
            wmQ = constp.tile([2 * C, HN], bf16, tag="wmQ")
            nc.sync.dma_start(wmQ[:], wmQ_d.ap())
            wmR = constp.tile([2 * C, HN], bf16, tag="wmR")
            nc.sync.dma_start(wmR[:], wmR_d.ap())
            nc.sync.dma_start(x2[:, XCUTS[1]:XCUTS[2]],
                              x_d.ap()[:, XCUTS[1]:XCUTS[2]])
            cw8 = constp.tile([8 * E, E], bf16, tag="cw8")
            nc.sync.dma_start(cw8[:], cw8_d.ap())
            cw1 = constp.tile([E + 1, E], bf16, tag="cw1")
            nc.sync.dma_start(cw1[:], cw1_d.ap())
            for q in range(2, len(XCUTS) - 1):
                a, b = XCUTS[q], XCUTS[q + 1]
                nc.sync.dma_start(x2[:, a:b], x_d.ap()[:, a:b])

            # --- HAM warm-up: bridge the PE from the preamble to the
            # first real matmul (~11us) so the stream starts at 2.4 GHz ---
            scr = constp.tile([128, 640], bf16, tag="scr")
            nc.vector.memset(scr[:], 0.0)
            zwarm = zpool.tile([128, 512], f32, name="zwarm", tag="zp")
            for _ in range(7):
                nc.tensor.matmul(zwarm[:], scr[:, 0:128], scr[:, 128:640],
                                 start=True, stop=True,
                                 skip_group_check=True)

            # --- main per-tile-PAIR loop ---
            # Tiles are processed in pairs with their full-K chunk runs,
            # conv2 smalls, and row-tiled pair-slot runs each contiguous:
            # the PE pays its ~95ns full<->row-tiled drain-serialization
            # penalty once per PAIR instead of twice per tile.
            def emit_full(h):
                c0 = h * WP
                zh = [zpool.tile([128, 512], f32, name="zh", tag="zp")
                      for _ in range(2)]
                for k in range(3):
                    lhsT = x2[:, c0 + k:c0 + k + 128]
                    for half in range(2):
                        sl = slice(HN * half, HN * half + HN)
                        nc.tensor.matmul(zh[half][:], lhsT, wmF[k][:, sl],
                                         start=(k == 0), stop=False,
                                         skip_group_check=True)
                return zh

            def emit_conv2(h):
                c0 = h * WP
                pp = ppool.tile([128, E], f32, name="pp", tag="pp")
                nc.tensor.matmul(pp[:], pc8[:, c0:c0 + 128], cw8[:],
                                 start=True, stop=False)
                nc.tensor.matmul(pp[:], pt8[:, c0:c0 + 128], cw1[:],
                                 start=False, stop=True)
                pa = workp.tile([128, E], f32, tag="pa")
                nc.scalar.activation(pa[:], pp[:], Copy)
                return pa

            def emit_pairs(h, zh):
                c0 = h * WP
                for (ot_, ob, wt, stp) in ((260, 131, wmP, False),
                                           (261, 130, wmQ, False),
                                           (262, 132, wmR, True)):
                    nc.tensor.matmul(zh[0][:], x2[0:C, c0 + ot_:c0 + ot_ + 128],
                                     wt[0:C, :], start=False, stop=stp,
                                     skip_group_check=True)
                    nc.tensor.matmul(zh[1][:], x2[C:2 * C, c0 + ob:c0 + ob + 128],
                                     wt[C:2 * C, :], start=False, stop=stp,
                                     skip_group_check=True)

            def emit_epi(h, zh, pa):
                m = workp.tile([128, E * C], bf16, tag="m", bufs=4)
                for e in range(NACT):
                    sl = slice(C * e, C * e + C)
                    nc.scalar.activation(m[:, sl], zh[0][:, sl], Copy,
                                         scale=pa[:, e:e + 1])
                zA = zh[0].rearrange("p (e o) -> p e o", o=C)[:, NACT:8, :]
                mm = m.rearrange("p (e o) -> p e o", o=C)
                paA = pa[:, NACT:8].broadcast_to((128, 8 - NACT, C))
                nc.vector.tensor_tensor(mm[:, NACT:8, :], zA, paA,
                                        mybir.AluOpType.mult)
                zB = zh[1].rearrange("p (e o) -> p e o", o=C)
                paB = pa[:, 8:E].broadcast_to((128, 8, C))
                nc.vector.tensor_tensor(mm[:, 8:E, :], zB, paB,
                                        mybir.AluOpType.mult)
                s1 = workp.tile([128, 512], bf16, tag="s1")
                nc.gpsimd.tensor_add(s1[:], m[:, 0:512], m[:, 512:1024])
                s2 = workp.tile([128, 256], bf16, tag="s2")
                nc.vector.tensor_add(s2[:], s1[:, 0:256], s1[:, 256:512])
                s3 = workp.tile([128, 128], bf16, tag="s3")
                nc.vector.tensor_add(s3[:], s2[:, 0:128], s2[:, 128:256])
                ot = workp.tile([128, C], bf16, tag="ot")
                nc.vector.tensor_add(ot[:], s3[:, 0:64], s3[:, 64:128])
                nc.sync.dma_start(
                    out_d.ap()[128 * h:128 * h + 128, :], ot[:])

            # z for tiles 0-1 BEFORE conv1: the tensor queue would
            # otherwise idle behind conv1 pairs that wait on late x2
            # chunks while tile 0 needs only chunk 0 + weights.
            zh_pre = [emit_full(0), emit_full(1)]

            # --- conv1 + relu(+pb) -> pb1 (16, L) bf16, row-tiled pairs ---
            pb1 = bigp.tile([E, L], bf16, tag="pb1")
            for i in range(8):
                a = 1024 * i
                pA = ppool.tile([E, 512], f32, name="p1a", tag="pp")
                nc.tensor.matmul(pA[:], pwm2[0:C, :], x2[0:C, a:a + 512],
                                 start=True, stop=True)
                pB = ppool.tile([E, 512], f32, name="p1b", tag="pp")
                nc.tensor.matmul(pB[:], pwm2[C:2 * C, :],
                                 x2[C:2 * C, a + 382:a + 894],
                                 start=True, stop=True)
                nc.scalar.activation(pb1[:, a:a + 512], pA[:], Relu,
                                     bias=pbv[:])
                nc.scalar.activation(pb1[:, a + 512:a + 1024], pB[:], Relu,
                                     bias=pbv[:])
            tail = L - 8192  # 388
            pT = ppool.tile([E, 512], f32, name="p1t", tag="pp")
            nc.tensor.matmul(pT[:, :tail], pwm2[0:C, :], x2[0:C, 8192:L],
                             start=True, stop=True)
            nc.scalar.activation(pb1[:, 8192:L], pT[:, :tail], Relu,
                                 bias=pbv[:])

            # --- predictor im2col: pc8 (8 taps) + pt8 (ones+tap8) ---
            pc8 = bigp.tile([8 * E, LC], bf16, tag="pc8")
            pt8 = bigp.tile([E + 1, LC], bf16, tag="pt8")
            offs = [dh * WP + dw for dh in (-1, 0, 1) for dw in (-1, 0, 1)]
            for bd in range(len(PCUTS) - 1):
                a, b = PCUTS[bd], PCUTS[bd + 1]
                n = b - a
                for t in range(8):
                    nc.sync.dma_start(
                        pc8[E * t:E * t + E, a:b],
                        pb1[:, a + 131 + offs[t]: a + 131 + offs[t] + n])
                nc.sync.dma_start(
                    pt8[1:E + 1, a:b],
                    pb1[:, a + 131 + offs[8]: a + 131 + offs[8] + n])
                nc.vector.memset(pt8[0:1, a:b], 1.0)

            pa0 = emit_conv2(0)
            pa1 = emit_conv2(1)
            emit_pairs(0, zh_pre[0])
            emit_pairs(1, zh_pre[1])
            emit_epi(0, zh_pre[0], pa0)
            emit_epi(1, zh_pre[1], pa1)
            for h in range(2, NT, 2):
                zh0 = emit_full(h)
                zh1 = emit_full(h + 1)
                pa0 = emit_conv2(h)
                pa1 = emit_conv2(h + 1)
                emit_pairs(h, zh0)
                emit_pairs(h + 1, zh1)
                emit_epi(h, zh0, pa0)
                emit_epi(h + 1, zh1, pa1)


    nc.compile()
    return nc


_CACHE = {}


def _get_nc():
    if "nc" not in _CACHE:
        _CACHE["nc"] = build_bass()
    return _CACHE["nc"]


def kernel(x, W, pw, pb, cw, cb):
    in_maps = build_core_inputs(
        dict(x=x, W=W, pw=pw, pb=pb, cw=cw, cb=cb))
    nc = _get_nc()
    res = run_bass_kernel_spmd(nc, in_maps, core_ids=list(range(NCORES)))
    return unshard_output(res.results)


# revision 34
# speedup vs baseline: 1.1451x; 1.1451x over previous
"""AdaConv (low-rank dynamic conv) on 8 trn2 NeuronCores.

out[b,o,h,w] = sum_e para[b,e,h,w] * conv3x3(x, W_e)[b,o,h,w]
  para = conv3x3(relu(conv1x1(x, pw) + pb), cw) + cb          (16 bases)
  W_e  = basis weight e reshaped from W (64 out, 64 in, 3x3)

Sharding: pure data parallel, 8 shards = (batch b, image half hh).
Each core computes out rows [hh*64, hh*64+64) of image b from x rows
[hh*64-1, hh*64+65) (1-row halo), zero padded at image borders.

Per-core kernel (pixel-partition layout, im2col fully by view):
  - host sends x2 (128, 66*130) bf16: rows 0-63 x (width-padded to 130),
    rows 64-127 the same image shifted by +130 (one padded row). A K=128
    matmul view at column offset o contracts taps (o-131, o-1) at once:
    offsets 0/1/2 cover the dh=-1 and dh=0 tap rows (6 taps, full K).
  - the dh=+1 taps (129/130/131) use K=64 row-tiled matmul PAIRS running
    concurrently on disjoint PE row groups: top rows (x) at offset
    131+tap serve one psum half while bottom rows (x shifted) at offset
    tap+1 serve the other; cross-pairing keeps both halves fed each
    slot, so 3 slots finish all 3 taps with no half-zero weight waste
    and no DMA-materialized im2col chunk.
  - conv1 also runs as row-tiled concurrent pairs (top half processes
    columns [a,a+512), bottom half [a+512,a+1024) via the +130 copy).
  - conv2 im2col: pc8 (8 taps x 16ch) + pt8 (ones row + tap8) of pb1,
    built by shifted SBUF->SBUF DMAs in 5 bands chasing conv1.
  - per output row h (128 pixels): z psum (128pix, 1024 eo) = 6 full-K
    matmuls + 3 row-tiled pair slots; para psum (128pix,16) from 2
    matmuls; epilogue multiplies z e-slabs by para columns (ScalarE
    2 slabs + VectorE 14) and tree-adds to bf16 out.
  - tiles run in FUSED PAIRS (full-K runs, conv2 smalls, and row-tiled
    runs each contiguous across both tiles): the PE charges ~95ns of
    drain serialization at every full<->row-tiled regime change, so
    pairing halves that cost; 3-tile groups exhaust the 6 z psum banks
    and stall.  Inputs stream on one queue in first-use order, z for
    tiles 0-1 is emitted before conv1 (whose late pairs wait on x2
    chunks), and 7 zero matmuls bridge the PE's HAM clock-gate from the
    preamble to the first real matmul so the stream starts at 2.4 GHz.
  - out written pixel-major (8192, 64) bf16; host transposes + casts.
"""

import numpy as np
import ml_dtypes

import concourse.bass as bass
import concourse.mybir as mybir
import concourse.tile as tile
from concourse import bacc
from concourse.bass_utils import run_bass_kernel_spmd

BF16 = ml_dtypes.bfloat16

B, C, H, WD = 4, 64, 128, 128
E = 16            # bases
NCORES = 8
HALF = H // 2     # 64 output rows per core
RH = HALF + 2     # 66 stored x rows (1 halo each side)
WP = WD + 2       # 130 padded width
L = RH * WP       # 8580 columns of the padded per-core image
LC = L - 2 * (WP + 1)  # 8318: im2col span, col i <-> center index i+131
NT = HALF         # 64 row-tiles per core, 128 pixels each
NACT = 2          # e-slabs handled by ScalarE in the epilogue
# tap order within a 3x3 kernel: t = (dh+1)*3 + (dw+1), offset = dh*130+dw

XCUTS = [0, 512, 1536, 3584, 5632, L]       # x2 DMA chunks (geometric-ish)
PCUTS = [0, 600, 2160, 4240, 6320, LC]      # pc8/pt8 build bands


def _prep_weights(Wt, pw, pb, cw, cb):
    """Host-side relayout of all weights (small, replicated to all cores)."""
    T = np.asarray(Wt, np.float32).reshape(C, C, 9, E)   # [o, c, tap, e]
    A = T.transpose(2, 1, 3, 0).reshape(9, C, E * C)     # [tap, c, (e*64+o)]
    HN = E * C // 2  # 512
    wm = {
        # full-K chunks: view o covers taps (o-131 via top, o-1 via bottom)
        "wm0": np.concatenate([A[0], A[3]], axis=0),   # taps -131, -1
        "wm1": np.concatenate([A[1], A[4]], axis=0),   # taps -130, 0
        "wm2": np.concatenate([A[2], A[5]], axis=0),   # taps -129, 1
        # row-tiled pair weights: [0:64]=top-row weights (half0 cols),
        # [64:128]=bottom-row weights (half1 cols)
        "wmP": np.concatenate([A[6][:, :HN], A[7][:, HN:]], axis=0),
        "wmQ": np.concatenate([A[7][:, :HN], A[6][:, HN:]], axis=0),
        "wmR": np.concatenate([A[8][:, :HN], A[8][:, HN:]], axis=0),
    }
    pwm = np.asarray(pw, np.float32).reshape(E, C).T          # (64, 16)
    pwm2 = np.concatenate([pwm, pwm], axis=0)                 # (128, 16)
    pbv = np.asarray(pb, np.float32).reshape(E, 1)            # (16, 1) f32

    A2 = np.asarray(cw, np.float32).transpose(2, 3, 1, 0).reshape(9, E, E)
    cw8 = A2[:8].reshape(8 * E, E)                            # (128, 16)
    # row 0 = cb (pairs with the ones row at partition 0 of pt8)
    cw1 = np.concatenate(
        [np.asarray(cb, np.float32).reshape(1, E), A2[8]], axis=0)  # (17, 16)

    out = {k: v.astype(BF16) for k, v in wm.items()}
    out.update(pwm2=pwm2.astype(BF16), pbv=np.asarray(pbv, np.float32),
               cw8=cw8.astype(BF16), cw1=cw1.astype(BF16))
    return out


def _shard_x(x):
    """(B,C,H,W) f32 -> 8 shards (128, L) bf16: padded image + copy at +130."""
    xp = np.zeros((B, C, H + 2, WP), np.float32)
    xp[:, :, 1:H + 1, 1:WD + 1] = x
    shards = []
    for b in range(B):
        for hh in range(2):
            rows = xp[b, :, hh * HALF: hh * HALF + RH, :].reshape(C, L)
            shifted = np.zeros_like(rows)
            shifted[:, :L - WP] = rows[:, WP:]
            shards.append(np.concatenate([rows, shifted], axis=0).astype(BF16))
    return shards


def build_core_inputs(inputs):
    """Full inputs dict -> list of per-core in_maps (host relayout)."""
    wts = _prep_weights(inputs["W"], inputs["pw"], inputs["pb"],
                        inputs["cw"], inputs["cb"])
    shards = _shard_x(np.asarray(inputs["x"], np.float32))
    return [dict(wts, x=shards[i]) for i in range(NCORES)]


def unshard_output(results):
    """Per-core result dicts -> full (B, C, H, W) f32 output."""
    out = np.empty((B, C, H, WD), np.float32)
    for i in range(NCORES):
        b, hh = divmod(i, 2)
        sh = np.asarray(results[i]["out"], np.float32)  # (8192, 64) pixel-major
        out[b, :, hh * HALF:(hh + 1) * HALF, :] = (
            sh.reshape(HALF, WD, C).transpose(2, 0, 1))
    return out


def build_bass():
    f32 = mybir.dt.float32
    bf16 = mybir.dt.bfloat16
    Relu = mybir.ActivationFunctionType.Relu
    Copy = mybir.ActivationFunctionType.Copy
    HN = E * C // 2  # 512

    nc = bacc.Bacc("TRN2", target_bir_lowering=False, debug=False,
                   num_devices=NCORES)

    x_d = nc.declare_dram_parameter("x", [2 * C, L], bf16, isOutput=False)
    wmF_d = [nc.declare_dram_parameter(f"wm{k}", [2 * C, E * C], bf16,
                                       isOutput=False) for k in range(3)]
    wmP_d = nc.declare_dram_parameter("wmP", [2 * C, HN], bf16, isOutput=False)
    wmQ_d = nc.declare_dram_parameter("wmQ", [2 * C, HN], bf16, isOutput=False)
    wmR_d = nc.declare_dram_parameter("wmR", [2 * C, HN], bf16, isOutput=False)
    pwm2_d = nc.declare_dram_parameter("pwm2", [2 * C, E], bf16, isOutput=False)
    pbv_d = nc.declare_dram_parameter("pbv", [E, 1], f32, isOutput=False)
    cw8_d = nc.declare_dram_parameter("cw8", [8 * E, E], bf16, isOutput=False)
    cw1_d = nc.declare_dram_parameter("cw1", [E + 1, E], bf16, isOutput=False)
    out_d = nc.declare_dram_parameter("out", [HALF * WD, C], bf16,
                                      isOutput=True)

    with tile.TileContext(nc) as tc:
        with (
            tc.tile_pool(name="const", bufs=1) as constp,
            tc.tile_pool(name="big", bufs=1) as bigp,
            tc.tile_pool(name="work", bufs=3) as workp,
            tc.tile_pool(name="ps_z", bufs=6, space="PSUM") as zpool,
            tc.tile_pool(name="ps_p", bufs=2, space="PSUM") as ppool,
        ):
            # --- input DMAs: one queue (sync) in first-use order so the
            # DMA engine pool delivers the z-critical path first ---
            x2 = bigp.tile([2 * C, L], bf16, tag="x2")
            pwm2 = constp.tile([2 * C, E], bf16, tag="pwm2")
            nc.sync.dma_start(pwm2[:], pwm2_d.ap())
            pbv = constp.tile([E, 1], f32, tag="pbv")
            nc.sync.dma_start(pbv[:], pbv_d.ap())
            nc.sync.dma_start(x2[:, XCUTS[0]:XCUTS[1]],
                              x_d.ap()[:, XCUTS[0]:XCUTS[1]])
            wmF = []
            for k in range(3):
                t = constp.tile([2 * C, E * C], bf16,
                                name=f"wm{k}s", tag=f"wm{k}")
                nc.sync.dma_start(t[:], wmF_d[k].ap())
                wmF.append(t)
            wmP = constp.tile([2 * C, HN], bf16, tag="wmP")
            nc.sync.dma_start(wmP[:], wmP_d.ap())
            wmQ = constp.tile([2 * C, HN], bf16, tag="wmQ")
            nc.sync.dma_start(wmQ[:], wmQ_d.ap())
            wmR = constp.tile([2 * C, HN], bf16, tag="wmR")
            nc.sync.dma_start(wmR[:], wmR_d.ap())
            nc.sync.dma_start(x2[:, XCUTS[1]:XCUTS[2]],
                              x_d.ap()[:, XCUTS[1]:XCUTS[2]])
            cw8 = constp.tile([8 * E, E], bf16, tag="cw8")
            nc.sync.dma_start(cw8[:], cw8_d.ap())
            cw1 = constp.tile([E + 1, E], bf16, tag="cw1")
            nc.sync.dma_start(cw1[:], cw1_d.ap())
            for q in range(2, len(XCUTS) - 1):
                a, b = XCUTS[q], XCUTS[q + 1]
                nc.sync.dma_start(x2[:, a:b], x_d.ap()[:, a:b])

            # --- HAM warm-up: bridge the PE from the preamble to the
            # first real matmul (~11us) so the stream starts at 2.4 GHz ---
            scr = constp.tile([128, 640], bf16, tag="scr")
            nc.vector.memset(scr[:], 0.0)
            zwarm = zpool.tile([128, 512], f32, name="zwarm", tag="zp")
            for _ in range(7):
                nc.tensor.matmul(zwarm[:], scr[:, 0:128], scr[:, 128:640],
                                 start=True, stop=True,
                                 skip_group_check=True)

            # --- main per-tile-PAIR loop ---
            # Tiles are processed in pairs with their full-K chunk runs,
            # conv2 smalls, and row-tiled pair-slot runs each contiguous:
            # the PE pays its ~95ns full<->row-tiled drain-serialization
            # penalty once per PAIR instead of twice per tile.
            def emit_full(h):
                c0 = h * WP
                zh = [zpool.tile([128, 512], f32, name="zh", tag="zp")
                      for _ in range(2)]
                for k in range(3):
                    lhsT = x2[:, c0 + k:c0 + k + 128]
                    for half in range(2):
                        sl = slice(HN * half, HN * half + HN)
                        nc.tensor.matmul(zh[half][:], lhsT, wmF[k][:, sl],
                                         start=(k == 0), stop=False,
                                         skip_group_check=True)
                return zh

            def emit_conv2(h):
                c0 = h * WP
                pp = ppool.tile([128, E], f32, name="pp", tag="pp")
                nc.tensor.matmul(pp[:], pc8[:, c0:c0 + 128], cw8[:],
                                 start=True, stop=False)
                nc.tensor.matmul(pp[:], pt8[:, c0:c0 + 128], cw1[:],
                                 start=False, stop=True)
                pa = workp.tile([128, E], f32, tag="pa")
                nc.scalar.activation(pa[:], pp[:], Copy)
                return pa

            def emit_pairs(h, zh):
                c0 = h * WP
                for (ot_, ob, wt, stp) in ((260, 131, wmP, False),
                                           (261, 130, wmQ, False),
                                           (262, 132, wmR, True)):
                    nc.tensor.matmul(zh[0][:], x2[0:C, c0 + ot_:c0 + ot_ + 128],
                                     wt[0:C, :], start=False, stop=stp,
                                     skip_group_check=True)
                    nc.tensor.matmul(zh[1][:], x2[C:2 * C, c0 + ob:c0 + ob + 128],
                                     wt[C:2 * C, :], start=False, stop=stp,
                                     skip_group_check=True)

            def emit_mults(h, zh, pa):
                m = workp.tile([128, E * C], bf16, tag="m", bufs=4)
                for e in range(NACT):
                    sl = slice(C * e, C * e + C)
                    nc.scalar.activation(m[:, sl], zh[0][:, sl], Copy,
                                         scale=pa[:, e:e + 1])
                zA = zh[0].rearrange("p (e o) -> p e o", o=C)[:, NACT:8, :]
                mm = m.rearrange("p (e o) -> p e o", o=C)
                paA = pa[:, NACT:8].broadcast_to((128, 8 - NACT, C))
                nc.vector.tensor_tensor(mm[:, NACT:8, :], zA, paA,
                                        mybir.AluOpType.mult)
                zB = zh[1].rearrange("p (e o) -> p e o", o=C)
                paB = pa[:, 8:E].broadcast_to((128, 8, C))
                nc.vector.tensor_tensor(mm[:, 8:E, :], zB, paB,
                                        mybir.AluOpType.mult)
                return m

            def emit_tree(h, m):
                s1 = workp.tile([128, 512], bf16, tag="s1")
                nc.vector.tensor_add(s1[:], m[:, 0:512], m[:, 512:1024])
                s2 = workp.tile([128, 256], bf16, tag="s2")
                nc.vector.tensor_add(s2[:], s1[:, 0:256], s1[:, 256:512])
                s3 = workp.tile([128, 128], bf16, tag="s3")
                nc.vector.tensor_add(s3[:], s2[:, 0:128], s2[:, 128:256])
                ot = workp.tile([128, C], bf16, tag="ot")
                nc.vector.tensor_add(ot[:], s3[:, 0:64], s3[:, 64:128])
                nc.sync.dma_start(
                    out_d.ap()[128 * h:128 * h + 128, :], ot[:])

            # z for tiles 0-1 BEFORE conv1: the tensor queue would
            # otherwise idle behind conv1 pairs that wait on late x2
            # chunks while tile 0 needs only chunk 0 + weights.
            zh_pre = [emit_full(0), emit_full(1)]

            # --- conv1 + relu(+pb) -> pb1 (16, L) bf16, row-tiled pairs ---
            pb1 = bigp.tile([E, L], bf16, tag="pb1")
            for i in range(8):
                a = 1024 * i
                pA = ppool.tile([E, 512], f32, name="p1a", tag="pp")
                nc.tensor.matmul(pA[:], pwm2[0:C, :], x2[0:C, a:a + 512],
                                 start=True, stop=True)
                pB = ppool.tile([E, 512], f32, name="p1b", tag="pp")
                nc.tensor.matmul(pB[:], pwm2[C:2 * C, :],
                                 x2[C:2 * C, a + 382:a + 894],
                                 start=True, stop=True)
                nc.scalar.activation(pb1[:, a:a + 512], pA[:], Relu,
                                     bias=pbv[:])
                nc.scalar.activation(pb1[:, a + 512:a + 1024], pB[:], Relu,
                                     bias=pbv[:])
            tail = L - 8192  # 388
            pT = ppool.tile([E, 512], f32, name="p1t", tag="pp")
            nc.tensor.matmul(pT[:, :tail], pwm2[0:C, :], x2[0:C, 8192:L],
                             start=True, stop=True)
            nc.scalar.activation(pb1[:, 8192:L], pT[:, :tail], Relu,
                                 bias=pbv[:])

            # --- predictor im2col: pc8 (8 taps) + pt8 (ones+tap8) ---
            pc8 = bigp.tile([8 * E, LC], bf16, tag="pc8")
            pt8 = bigp.tile([E + 1, LC], bf16, tag="pt8")
            offs = [dh * WP + dw for dh in (-1, 0, 1) for dw in (-1, 0, 1)]
            for bd in range(len(PCUTS) - 1):
                a, b = PCUTS[bd], PCUTS[bd + 1]
                n = b - a
                for t in range(8):
                    nc.sync.dma_start(
                        pc8[E * t:E * t + E, a:b],
                        pb1[:, a + 131 + offs[t]: a + 131 + offs[t] + n])
                nc.sync.dma_start(
                    pt8[1:E + 1, a:b],
                    pb1[:, a + 131 + offs[8]: a + 131 + offs[8] + n])
                nc.vector.memset(pt8[0:1, a:b], 1.0)

            pa0 = emit_conv2(0)
            pa1 = emit_conv2(1)
            emit_pairs(0, zh_pre[0])
            emit_pairs(1, zh_pre[1])
            m0 = emit_mults(0, zh_pre[0], pa0)
            m1 = emit_mults(1, zh_pre[1], pa1)
            emit_tree(0, m0)
            emit_tree(1, m1)
            for h in range(2, NT, 2):
                zh0 = emit_full(h)
                zh1 = emit_full(h + 1)
                pa0 = emit_conv2(h)
                pa1 = emit_conv2(h + 1)
                emit_pairs(h, zh0)
                emit_pairs(h + 1, zh1)
                m0 = emit_mults(h, zh0, pa0)
                m1 = emit_mults(h + 1, zh1, pa1)
                emit_tree(h, m0)
                emit_tree(h + 1, m1)


    nc.compile()
    return nc


_CACHE = {}


def _get_nc():
    if "nc" not in _CACHE:
        _CACHE["nc"] = build_bass()
    return _CACHE["nc"]


def kernel(x, W, pw, pb, cw, cb):
    in_maps = build_core_inputs(
        dict(x=x, W=W, pw=pw, pb=pb, cw=cw, cb=cb))
    nc = _get_nc()
    res = run_bass_kernel_spmd(nc, in_maps, core_ids=list(range(NCORES)))
    return unshard_output(res.results)


# revision 35
# speedup vs baseline: 1.1700x; 1.0217x over previous
"""AdaConv (low-rank dynamic conv) on 8 trn2 NeuronCores.

out[b,o,h,w] = sum_e para[b,e,h,w] * conv3x3(x, W_e)[b,o,h,w]
  para = conv3x3(relu(conv1x1(x, pw) + pb), cw) + cb          (16 bases)
  W_e  = basis weight e reshaped from W (64 out, 64 in, 3x3)

Sharding: pure data parallel, 8 shards = (batch b, image half hh).
Each core computes out rows [hh*64, hh*64+64) of image b from x rows
[hh*64-1, hh*64+65) (1-row halo), zero padded at image borders.

Per-core kernel (pixel-partition layout, im2col fully by view):
  - host sends x2 (128, 66*130) bf16: rows 0-63 x (width-padded to 130),
    rows 64-127 the same image shifted by +130 (one padded row). A K=128
    matmul view at column offset o contracts taps (o-131, o-1) at once:
    offsets 0/1/2 cover the dh=-1 and dh=0 tap rows (6 taps, full K).
  - the dh=+1 taps (129/130/131) use K=64 row-tiled matmul PAIRS running
    concurrently on disjoint PE row groups: top rows (x) at offset
    131+tap serve one psum half while bottom rows (x shifted) at offset
    tap+1 serve the other; cross-pairing keeps both halves fed each
    slot, so 3 slots finish all 3 taps with no half-zero weight waste
    and no DMA-materialized im2col chunk.
  - conv1 also runs as row-tiled concurrent pairs (top half processes
    columns [a,a+512), bottom half [a+512,a+1024) via the +130 copy).
  - conv2 im2col: pc8 (8 taps x 16ch) + pt8 (ones row + tap8) of pb1,
    built by shifted SBUF->SBUF DMAs in 5 bands chasing conv1.
  - per output row h (128 pixels): z psum (128pix, 1024 eo) = 6 full-K
    matmuls + 3 row-tiled pair slots; para psum (128pix,16) from 2
    matmuls; epilogue multiplies z e-slabs by para columns (ScalarE
    2 slabs + VectorE 14) and tree-adds to bf16 out.
  - tiles run in FUSED PAIRS (full-K runs, conv2 smalls, and row-tiled
    runs each contiguous across both tiles): the PE charges ~95ns of
    drain serialization at every full<->row-tiled regime change, so
    pairing halves that cost; 3-tile groups exhaust the 6 z psum banks
    and stall.  Inputs stream on one queue in first-use order, z for
    tiles 0-1 is emitted before conv1 (whose late pairs wait on x2
    chunks), and 7 zero matmuls bridge the PE's HAM clock-gate from the
    preamble to the first real matmul so the stream starts at 2.4 GHz.
  - out written pixel-major (8192, 64) bf16; host transposes + casts.
"""

import numpy as np
import ml_dtypes

import concourse.bass as bass
import concourse.mybir as mybir
import concourse.tile as tile
from concourse import bacc
from concourse.bass_utils import run_bass_kernel_spmd

BF16 = ml_dtypes.bfloat16

B, C, H, WD = 4, 64, 128, 128
E = 16            # bases
NCORES = 8
HALF = H // 2     # 64 output rows per core
RH = HALF + 2     # 66 stored x rows (1 halo each side)
WP = WD + 2       # 130 padded width
L = RH * WP       # 8580 columns of the padded per-core image
LC = L - 2 * (WP + 1)  # 8318: im2col span, col i <-> center index i+131
NT = HALF         # 64 row-tiles per core, 128 pixels each
NACT = 2          # e-slabs handled by ScalarE in the epilogue
# tap order within a 3x3 kernel: t = (dh+1)*3 + (dw+1), offset = dh*130+dw

XCUTS = [0, 512, 1536, 3584, 5632, L]       # x2 DMA chunks (geometric-ish)
PCUTS = [0, 600, 2160, 4240, 6320, LC]      # pc8/pt8 build bands


def _prep_weights(Wt, pw, pb, cw, cb):
    """Host-side relayout of all weights (small, replicated to all cores)."""
    T = np.asarray(Wt, np.float32).reshape(C, C, 9, E)   # [o, c, tap, e]
    A = T.transpose(2, 1, 3, 0).reshape(9, C, E * C)     # [tap, c, (e*64+o)]
    HN = E * C // 2  # 512
    wm = {
        # full-K chunks: view o covers taps (o-131 via top, o-1 via bottom)
        "wm0": np.concatenate([A[0], A[3]], axis=0),   # taps -131, -1
        "wm1": np.concatenate([A[1], A[4]], axis=0),   # taps -130, 0
        "wm2": np.concatenate([A[2], A[5]], axis=0),   # taps -129, 1
        # row-tiled pair weights: [0:64]=top-row weights (half0 cols),
        # [64:128]=bottom-row weights (half1 cols)
        "wmP": np.concatenate([A[6][:, :HN], A[7][:, HN:]], axis=0),
        "wmQ": np.concatenate([A[7][:, :HN], A[6][:, HN:]], axis=0),
        "wmR": np.concatenate([A[8][:, :HN], A[8][:, HN:]], axis=0),
    }
    pwm = np.asarray(pw, np.float32).reshape(E, C).T          # (64, 16)
    pwm2 = np.concatenate([pwm, pwm], axis=0)                 # (128, 16)
    pbv = np.asarray(pb, np.float32).reshape(E, 1)            # (16, 1) f32

    A2 = np.asarray(cw, np.float32).transpose(2, 3, 1, 0).reshape(9, E, E)
    cw8 = A2[:8].reshape(8 * E, E)                            # (128, 16)
    # row 0 = cb (pairs with the ones row at partition 0 of pt8)
    cw1 = np.concatenate(
        [np.asarray(cb, np.float32).reshape(1, E), A2[8]], axis=0)  # (17, 16)

    out = {k: v.astype(BF16) for k, v in wm.items()}
    out.update(pwm2=pwm2.astype(BF16), pbv=np.asarray(pbv, np.float32),
               cw8=cw8.astype(BF16), cw1=cw1.astype(BF16))
    return out


def _shard_x(x):
    """(B,C,H,W) f32 -> 8 shards (128, L) bf16: padded image + copy at +130."""
    xp = np.zeros((B, C, H + 2, WP), np.float32)
    xp[:, :, 1:H + 1, 1:WD + 1] = x
    shards = []
    for b in range(B):
        for hh in range(2):
            rows = xp[b, :, hh * HALF: hh * HALF + RH, :].reshape(C, L)
            shifted = np.zeros_like(rows)
            shifted[:, :L - WP] = rows[:, WP:]
            shards.append(np.concatenate([rows, shifted], axis=0).astype(BF16))
    return shards


def build_core_inputs(inputs):
    """Full inputs dict -> list of per-core in_maps (host relayout)."""
    wts = _prep_weights(inputs["W"], inputs["pw"], inputs["pb"],
                        inputs["cw"], inputs["cb"])
    shards = _shard_x(np.asarray(inputs["x"], np.float32))
    return [dict(wts, x=shards[i]) for i in range(NCORES)]


def unshard_output(results):
    """Per-core result dicts -> full (B, C, H, W) f32 output."""
    out = np.empty((B, C, H, WD), np.float32)
    for i in range(NCORES):
        b, hh = divmod(i, 2)
        sh = np.asarray(results[i]["out"], np.float32)  # (8192, 64) pixel-major
        out[b, :, hh * HALF:(hh + 1) * HALF, :] = (
            sh.reshape(HALF, WD, C).transpose(2, 0, 1))
    return out


def build_bass():
    f32 = mybir.dt.float32
    bf16 = mybir.dt.bfloat16
    Relu = mybir.ActivationFunctionType.Relu
    Copy = mybir.ActivationFunctionType.Copy
    HN = E * C // 2  # 512

    nc = bacc.Bacc("TRN2", target_bir_lowering=False, debug=False,
                   num_devices=NCORES)

    x_d = nc.declare_dram_parameter("x", [2 * C, L], bf16, isOutput=False)
    wmF_d = [nc.declare_dram_parameter(f"wm{k}", [2 * C, E * C], bf16,
                                       isOutput=False) for k in range(3)]
    wmP_d = nc.declare_dram_parameter("wmP", [2 * C, HN], bf16, isOutput=False)
    wmQ_d = nc.declare_dram_parameter("wmQ", [2 * C, HN], bf16, isOutput=False)
    wmR_d = nc.declare_dram_parameter("wmR", [2 * C, HN], bf16, isOutput=False)
    pwm2_d = nc.declare_dram_parameter("pwm2", [2 * C, E], bf16, isOutput=False)
    pbv_d = nc.declare_dram_parameter("pbv", [E, 1], f32, isOutput=False)
    cw8_d = nc.declare_dram_parameter("cw8", [8 * E, E], bf16, isOutput=False)
    cw1_d = nc.declare_dram_parameter("cw1", [E + 1, E], bf16, isOutput=False)
    out_d = nc.declare_dram_parameter("out", [HALF * WD, C], bf16,
                                      isOutput=True)

    with tile.TileContext(nc) as tc:
        with (
            tc.tile_pool(name="const", bufs=1) as constp,
            tc.tile_pool(name="big", bufs=1) as bigp,
            tc.tile_pool(name="work", bufs=3) as workp,
            tc.tile_pool(name="ps_z", bufs=6, space="PSUM") as zpool,
            tc.tile_pool(name="ps_p", bufs=2, space="PSUM") as ppool,
        ):
            # --- input DMAs: one queue (sync) in first-use order so the
            # DMA engine pool delivers the z-critical path first ---
            x2 = bigp.tile([2 * C, L], bf16, tag="x2")
            pwm2 = constp.tile([2 * C, E], bf16, tag="pwm2")
            nc.sync.dma_start(pwm2[:], pwm2_d.ap())
            pbv = constp.tile([E, 1], f32, tag="pbv")
            nc.sync.dma_start(pbv[:], pbv_d.ap())
            nc.sync.dma_start(x2[:, XCUTS[0]:XCUTS[1]],
                              x_d.ap()[:, XCUTS[0]:XCUTS[1]])
            wmF = []
            for k in range(3):
                t = constp.tile([2 * C, E * C], bf16,
                                name=f"wm{k}s", tag=f"wm{k}")
                nc.sync.dma_start(t[:], wmF_d[k].ap())
                wmF.append(t)
            wmP = constp.tile([2 * C, HN], bf16, tag="wmP")
            nc.sync.dma_start(wmP[:], wmP_d.ap())
            wmQ = constp.tile([2 * C, HN], bf16, tag="wmQ")
            nc.sync.dma_start(wmQ[:], wmQ_d.ap())
            wmR = constp.tile([2 * C, HN], bf16, tag="wmR")
            nc.sync.dma_start(wmR[:], wmR_d.ap())
            nc.sync.dma_start(x2[:, XCUTS[1]:XCUTS[2]],
                              x_d.ap()[:, XCUTS[1]:XCUTS[2]])
            cw8 = constp.tile([8 * E, E], bf16, tag="cw8")
            nc.sync.dma_start(cw8[:], cw8_d.ap())
            cw1 = constp.tile([E + 1, E], bf16, tag="cw1")
            nc.sync.dma_start(cw1[:], cw1_d.ap())
            for q in range(2, len(XCUTS) - 1):
                a, b = XCUTS[q], XCUTS[q + 1]
                nc.sync.dma_start(x2[:, a:b], x_d.ap()[:, a:b])

            # --- HAM warm-up: bridge the PE from the preamble to the
            # first real matmul (~11us) so the stream starts at 2.4 GHz ---
            scr = constp.tile([128, 640], bf16, tag="scr")
            nc.vector.memset(scr[:], 0.0)
            zwarm = zpool.tile([128, 512], f32, name="zwarm", tag="zp")
            for _ in range(7):
                nc.tensor.matmul(zwarm[:], scr[:, 0:128], scr[:, 128:640],
                                 start=True, stop=True,
                                 skip_group_check=True)

            # --- main per-tile-PAIR loop ---
            # Tiles are processed in pairs with their full-K chunk runs,
            # conv2 smalls, and row-tiled pair-slot runs each contiguous:
            # the PE pays its ~95ns full<->row-tiled drain-serialization
            # penalty once per PAIR instead of twice per tile.
            def emit_full(h):
                c0 = h * WP
                zh = [zpool.tile([128, 512], f32, name="zh", tag="zp")
                      for _ in range(2)]
                for k in range(3):
                    lhsT = x2[:, c0 + k:c0 + k + 128]
                    for half in range(2):
                        sl = slice(HN * half, HN * half + HN)
                        nc.tensor.matmul(zh[half][:], lhsT, wmF[k][:, sl],
                                         start=(k == 0), stop=False,
                                         skip_group_check=True)
                return zh

            def emit_conv2a(h):
                c0 = h * WP
                pp = ppool.tile([128, E], f32, name="pp", tag="pp")
                nc.tensor.matmul(pp[:], pc8[:, c0:c0 + 128], cw8[:],
                                 start=True, stop=False)
                return pp

            def emit_conv2b(h, pp):
                c0 = h * WP
                nc.tensor.matmul(pp[:], pt8[:, c0:c0 + 128], cw1[:],
                                 start=False, stop=True)
                pa = workp.tile([128, E], f32, tag="pa")
                nc.scalar.activation(pa[:], pp[:], Copy)
                return pa

            def emit_pairs(h, zh):
                c0 = h * WP
                for (ot_, ob, wt, stp) in ((260, 131, wmP, False),
                                           (261, 130, wmQ, False),
                                           (262, 132, wmR, True)):
                    nc.tensor.matmul(zh[0][:], x2[0:C, c0 + ot_:c0 + ot_ + 128],
                                     wt[0:C, :], start=False, stop=stp,
                                     skip_group_check=True)
                    nc.tensor.matmul(zh[1][:], x2[C:2 * C, c0 + ob:c0 + ob + 128],
                                     wt[C:2 * C, :], start=False, stop=stp,
                                     skip_group_check=True)

            def emit_mults(h, zh, pa):
                m = workp.tile([128, E * C], bf16, tag="m", bufs=4)
                for e in range(NACT):
                    sl = slice(C * e, C * e + C)
                    nc.scalar.activation(m[:, sl], zh[0][:, sl], Copy,
                                         scale=pa[:, e:e + 1])
                zA = zh[0].rearrange("p (e o) -> p e o", o=C)[:, NACT:8, :]
                mm = m.rearrange("p (e o) -> p e o", o=C)
                paA = pa[:, NACT:8].broadcast_to((128, 8 - NACT, C))
                nc.vector.tensor_tensor(mm[:, NACT:8, :], zA, paA,
                                        mybir.AluOpType.mult)
                zB = zh[1].rearrange("p (e o) -> p e o", o=C)
                paB = pa[:, 8:E].broadcast_to((128, 8, C))
                nc.vector.tensor_tensor(mm[:, 8:E, :], zB, paB,
                                        mybir.AluOpType.mult)
                return m

            def emit_tree(h, m):
                s1 = workp.tile([128, 512], bf16, tag="s1")
                nc.vector.tensor_add(s1[:], m[:, 0:512], m[:, 512:1024])
                s2 = workp.tile([128, 256], bf16, tag="s2")
                nc.vector.tensor_add(s2[:], s1[:, 0:256], s1[:, 256:512])
                s3 = workp.tile([128, 128], bf16, tag="s3")
                nc.vector.tensor_add(s3[:], s2[:, 0:128], s2[:, 128:256])
                ot = workp.tile([128, C], bf16, tag="ot")
                nc.vector.tensor_add(ot[:], s3[:, 0:64], s3[:, 64:128])
                nc.sync.dma_start(
                    out_d.ap()[128 * h:128 * h + 128, :], ot[:])

            # z for tiles 0-1 BEFORE conv1: the tensor queue would
            # otherwise idle behind conv1 pairs that wait on late x2
            # chunks while tile 0 needs only chunk 0 + weights.
            zh_pre = [emit_full(0), emit_full(1)]

            # --- conv1 + relu(+pb) -> pb1 (16, L) bf16, row-tiled pairs ---
            pb1 = bigp.tile([E, L], bf16, tag="pb1")
            for i in range(8):
                a = 1024 * i
                pA = ppool.tile([E, 512], f32, name="p1a", tag="pp")
                nc.tensor.matmul(pA[:], pwm2[0:C, :], x2[0:C, a:a + 512],
                                 start=True, stop=True)
                pB = ppool.tile([E, 512], f32, name="p1b", tag="pp")
                nc.tensor.matmul(pB[:], pwm2[C:2 * C, :],
                                 x2[C:2 * C, a + 382:a + 894],
                                 start=True, stop=True)
                nc.scalar.activation(pb1[:, a:a + 512], pA[:], Relu,
                                     bias=pbv[:])
                nc.scalar.activation(pb1[:, a + 512:a + 1024], pB[:], Relu,
                                     bias=pbv[:])
            tail = L - 8192  # 388
            pT = ppool.tile([E, 512], f32, name="p1t", tag="pp")
            nc.tensor.matmul(pT[:, :tail], pwm2[0:C, :], x2[0:C, 8192:L],
                             start=True, stop=True)
            nc.scalar.activation(pb1[:, 8192:L], pT[:, :tail], Relu,
                                 bias=pbv[:])

            # --- predictor im2col: pc8 (8 taps) + pt8 (ones+tap8) ---
            pc8 = bigp.tile([8 * E, LC], bf16, tag="pc8")
            pt8 = bigp.tile([E + 1, LC], bf16, tag="pt8")
            offs = [dh * WP + dw for dh in (-1, 0, 1) for dw in (-1, 0, 1)]
            for bd in range(len(PCUTS) - 1):
                a, b = PCUTS[bd], PCUTS[bd + 1]
                n = b - a
                for t in range(8):
                    nc.sync.dma_start(
                        pc8[E * t:E * t + E, a:b],
                        pb1[:, a + 131 + offs[t]: a + 131 + offs[t] + n])
                nc.sync.dma_start(
                    pt8[1:E + 1, a:b],
                    pb1[:, a + 131 + offs[8]: a + 131 + offs[8] + n])
                nc.vector.memset(pt8[0:1, a:b], 1.0)

            pp0 = emit_conv2a(0)
            pp1 = emit_conv2a(1)
            pa0 = emit_conv2b(0, pp0)
            pa1 = emit_conv2b(1, pp1)
            emit_pairs(0, zh_pre[0])
            emit_pairs(1, zh_pre[1])
            m0 = emit_mults(0, zh_pre[0], pa0)
            m1 = emit_mults(1, zh_pre[1], pa1)
            emit_tree(0, m0)
            emit_tree(1, m1)
            for h in range(2, NT, 2):
                zh0 = emit_full(h)
                zh1 = emit_full(h + 1)
                pp0 = emit_conv2a(h)
                pp1 = emit_conv2a(h + 1)
                pa0 = emit_conv2b(h, pp0)
                pa1 = emit_conv2b(h + 1, pp1)
                emit_pairs(h, zh0)
                emit_pairs(h + 1, zh1)
                m0 = emit_mults(h, zh0, pa0)
                m1 = emit_mults(h + 1, zh1, pa1)
                emit_tree(h, m0)
                emit_tree(h + 1, m1)


    nc.compile()
    return nc


_CACHE = {}


def _get_nc():
    if "nc" not in _CACHE:
        _CACHE["nc"] = build_bass()
    return _CACHE["nc"]


def kernel(x, W, pw, pb, cw, cb):
    in_maps = build_core_inputs(
        dict(x=x, W=W, pw=pw, pb=pb, cw=cw, cb=cb))
    nc = _get_nc()
    res = run_bass_kernel_spmd(nc, in_maps, core_ids=list(range(NCORES)))
    return unshard_output(res.results)
